# revision 22
# baseline (speedup 1.0000x reference)
"""Trainium2 Bass kernel for IntervalClusterTriplet (hard-mining triplet loss).

Math: loss = mean_i relu(sqrt(max_{j in cluster(i)} d2_ij)
                       - sqrt(min_{j not in cluster(i)} d2_ij) + 1)
with d2_ij = n_i + n_j - 2 G_ij. Only the max/min *values* are needed.

Design (vs the rank-1 fp32r baseline, 313us single-shot):
  - fp16 inputs: half the HBM traffic, 1 cyc/row matmuls. The PE runs
    at the cold ~0.8 GHz clock for a single-shot execution (the DVFS
    boost timescale exceeds the kernel), so total PE work is minimized,
    not just instruction count.
  - The gram is em2^T @ et (em2 = -2*et). The n_j bias is added by the
    cheapest available engine per pair ("routes"): 'A' pairs carry it as
    a K=1 rank-1 matmul (rhs = row 0 of the broadcast norms tile nb16)
    and the DVE reduces straight from fp32 PSUM; 'C' pairs are cast to
    fp16 SBUF by ScalarE, biased by GPSIMD (no PSUM port there), and
    min-reduced by the DVE at the lower SBUF access cost. The A/C split
    balances PE vs DVE/Act/Pool occupancy; all four engines land at
    ~60-80% busy.
  - 1024-wide paired PSUM tiles (2 banks) amortize per-instruction
    overheads; 3 pair slots + 2 single banks fill PSUM exactly.
  - Cluster masks are additive fp16 windows applied by GPSIMD on the
    cast diag pair, off the DVE critical path; the positive (in-cluster
    max) needs only the 128-wide diag block.

Sharding: rows of the distance matrix across 8 cores (1024 rows each).
Each core receives E^T rolled so its own 1024 columns come first (one
SPMD program). Per-core output is the partial loss sum; host adds the
8 scalars and divides by N.
"""

import numpy as np

import concourse.bacc as bacc
import concourse.mybir as mybir
import concourse.tile as tile
from concourse.bass_utils import run_bass_kernel_spmd

C, S, D = 1024, 8, 128
N = C * S              # 8192 embeddings
CORES = 8
M = N // CORES         # 1024 rows per core
P = 128                # partitions (rows per chunk)
CH = M // P            # 8 chunks per core
TN = 512               # column tile (one PSUM bank)
NT = N // TN           # 16 column tiles
PAIR = 2 * TN          # 1024-wide paired PSUM tile (2 banks)
NPAIR = N // PAIR      # 8 pairs
BIG = 3.0e38
MASKF = 60000.0        # fp16-safe mask magnitude
F32 = mybir.dt.float32
F16 = mybir.dt.float16
ALU = mybir.AluOpType
AX = mybir.AxisListType
ACT = mybir.ActivationFunctionType

# Route per pair 1..7 (pair 0 gets special diag handling on the C path):
#   'A' = K=1 rank-1 bias matmul rides the gram; DVE reduces straight
#         from PSUM (fp32). Cheapest total work, costs PE time.
#   'C' = ScalarE fp16 cast + GPSIMD bias add + DVE fp16 reduce.
#   'D' = DVE bias add direct from PSUM (fp32 -> fp16) + DVE fp16 reduce.
ROUTES_EVEN = ("A", "C", "A", "C", "C", "A", "C")
ROUTES_ODD = ("A", "C", "A", "C", "C", "A", "C")

_CACHE: dict = {}


def build_program(reps: int = 1):
    nc = bacc.Bacc("TRN2", target_bir_lowering=False, debug=False)
    et_d = nc.dram_tensor("et", [D, N], F16, kind="ExternalInput").ap()
    mwin_d = nc.dram_tensor("mwin", [P, 4 * TN], F16, kind="ExternalInput").ap()
    psel_d = nc.dram_tensor("psel", [P, P], F16, kind="ExternalInput").ap()
    oneh_d = nc.dram_tensor("oneh", [P, P], F16, kind="ExternalInput").ap()
    oner_d = nc.dram_tensor("oner", [1, P], F16, kind="ExternalInput").ap()
    ones2h_d = nc.dram_tensor("ones2h", [P, 2], F16, kind="ExternalInput").ap()
    ones2f_d = nc.dram_tensor("ones2f", [P, 2], F32, kind="ExternalInput").ap()
    out_d = nc.dram_tensor("out", [1, 1], F32, kind="ExternalOutput").ap()

    def body(tc, const, work, g16p, small, pbig, psmall):
        # ---- input DMAs: 8 pair-wide column chunks spread across queues
        et = const.tile([D, N], F16, tag="et")
        for pi in range(NPAIR):
            sl = slice(pi * PAIR, (pi + 1) * PAIR)
            nc.sync.dma_start(et[:, sl], et_d[:, sl])
        mwin = const.tile([P, 4 * TN], F16, tag="mwin")
        nc.sync.dma_start(mwin, mwin_d)
        psel = const.tile([P, P], F16, tag="psel")
        nc.sync.dma_start(psel, psel_d)
        oneh = const.tile([P, P], F16, tag="oneh")
        nc.sync.dma_start(oneh, oneh_d)
        oner = const.tile([1, P], F16, tag="oner")
        nc.sync.dma_start(oner, oner_d)
        ones2h = const.tile([P, 2], F16, tag="ones2h")
        nc.sync.dma_start(ones2h, ones2h_d)
        ones2f = const.tile([P, 2], F32, tag="ones2f")
        nc.sync.dma_start(ones2f, ones2f_d)

        # ---- esq = et^2 (ScalarE square, off the DVE); em2 = -2*et (DVE)
        esq = work.tile([D, N], F16, tag="esq")
        for pi in range(NPAIR):
            sl = slice(pi * PAIR, (pi + 1) * PAIR)
            nc.scalar.activation(esq[:, sl], et[:, sl], ACT.Square)
        em2 = work.tile([D, M], F16, tag="em2")
        nc.vector.tensor_scalar_mul(em2, et[:, 0:M], -2.0)

        # ---- nb16[p, j] = n_j for all partitions (ones matmul), cast to
        # fp16 SBUF by ScalarE. Row 0 doubles as the rank-1 rhs.
        nb16 = work.tile([P, N], F16, tag="nb16")
        for pi in range(NPAIR):
            pnb = pbig.tile([P, PAIR], F32, tag="pt")
            for h in range(2):
                t = 2 * pi + h
                nc.tensor.matmul(pnb[:, h * TN:(h + 1) * TN], lhsT=oneh,
                                 rhs=esq[:, t * TN:(t + 1) * TN],
                                 start=True, stop=True)
            nc.scalar.copy(nb16[:, pi * PAIR:(pi + 1) * PAIR], pnb)

        # ---- nmy[:, m] = n_i for my rows (chunk m)
        nmy = work.tile([P, CH], F32, tag="nmy")
        for m in range(CH):
            pm = psmall.tile([P, 2], F32, tag="pm")
            nc.tensor.matmul(pm, lhsT=esq[:, m * P:(m + 1) * P],
                             rhs=ones2h, start=True, stop=True)
            nc.scalar.copy(nmy[:, m:m + 1], pm[:, 0:1])

        losses = work.tile([P, CH], F32, tag="losses")

        # ---- main loop: 8 chunks x 8 pairs (grams only on the PE; the
        # n_j bias is added post-PSUM by GPSIMD (C) or DVE (D))
        for m in range(CH):
            td, off = m // 4, (m % 4) * P
            routes = ROUTES_EVEN if m % 2 == 0 else ROUTES_ODD
            # min contributions, all in units of n_j - 2G:
            # [0]=pair0 normal half, [1]=pair0 masked diag half,
            # [2..9)=pairs 1..7 (any route)
            mincols = small.tile([P, 9], F32, tag="mincols")
            apm = small.tile([P, 1], F32, tag="apm")
            for pi in range(NPAIR):
                route = None if pi == 0 else routes[pi - 1]
                pt = pbig.tile([P, PAIR], F32, tag="pt")
                for h in range(2):
                    t = 2 * pi + h
                    hs = slice(h * TN, (h + 1) * TN)
                    ts = slice(t * TN, (t + 1) * TN)
                    if route == "A":
                        nc.tensor.matmul(pt[:, hs], lhsT=em2[:, m * P:(m + 1) * P],
                                         rhs=et[:, ts], start=True, stop=False)
                        nc.tensor.matmul(pt[:, hs], lhsT=oner,
                                         rhs=nb16[0:1, ts], start=False, stop=True)
                    else:
                        nc.tensor.matmul(pt[:, hs], lhsT=em2[:, m * P:(m + 1) * P],
                                         rhs=et[:, ts], start=True, stop=True)
                psl = slice(pi * PAIR, (pi + 1) * PAIR)
                if pi == 0:
                    # cast; bias + diag masks via GPSIMD additive tiles
                    g = g16p.tile([P, PAIR], F16, tag="g16")
                    nc.scalar.copy(g, pt)
                    nd = 1 - td
                    nc.gpsimd.tensor_add(g, g, nb16[:, psl])
                    # positive: cluster-masked max over the 128-block
                    pb = small.tile([P, P], F16, tag="pb")
                    nc.gpsimd.tensor_add(pb, g[:, td * TN + off:td * TN + off + P],
                                         psel)
                    # negative: +MASKF window over the diag half (in place)
                    nc.gpsimd.tensor_add(g[:, td * TN:(td + 1) * TN],
                                         g[:, td * TN:(td + 1) * TN],
                                         mwin[:, (m % 4) * TN:(m % 4 + 1) * TN])
                    nc.vector.tensor_reduce(mincols[:, 0:1],
                                            g[:, nd * TN:(nd + 1) * TN],
                                            axis=AX.X, op=ALU.min)
                    nc.vector.tensor_reduce(mincols[:, 1:2],
                                            g[:, td * TN:(td + 1) * TN],
                                            axis=AX.X, op=ALU.min)
                    nc.vector.tensor_reduce(apm, pb, axis=AX.X, op=ALU.max)
                elif route == "A":  # bias already in PSUM via rank-1
                    nc.vector.tensor_reduce(mincols[:, pi + 1:pi + 2], pt,
                                            axis=AX.X, op=ALU.min)
                elif route == "C":
                    g = g16p.tile([P, PAIR], F16, tag="g16")
                    nc.scalar.copy(g, pt)
                    h16 = g16p.tile([P, PAIR], F16, tag="h16")
                    nc.gpsimd.tensor_add(h16, g, nb16[:, psl])
                    nc.vector.tensor_reduce(mincols[:, pi + 1:pi + 2], h16,
                                            axis=AX.X, op=ALU.min)
                else:  # 'D': DVE adds the bias straight from PSUM
                    h16 = g16p.tile([P, PAIR], F16, tag="h16")
                    nc.vector.tensor_add(h16, pt, nb16[:, psl])
                    nc.vector.tensor_reduce(mincols[:, pi + 1:pi + 2], h16,
                                            axis=AX.X, op=ALU.min)
            # epilogue: ap/an -> hinge loss for this chunk's 128 rows
            anm = small.tile([P, 1], F32, tag="anm")
            nc.vector.tensor_reduce(anm, mincols, axis=AX.X, op=ALU.min)
            apsq = small.tile([P, 1], F32, tag="apsq")
            nc.vector.tensor_scalar(apsq, apm, nmy[:, m:m + 1], 0.0,
                                    op0=ALU.add, op1=ALU.max)
            ansq = small.tile([P, 1], F32, tag="ansq")
            nc.vector.tensor_scalar(ansq, anm, nmy[:, m:m + 1], 0.0,
                                    op0=ALU.add, op1=ALU.max)
            ap = small.tile([P, 1], F32, tag="ap")
            nc.scalar.activation(ap, apsq, ACT.Sqrt)
            an = small.tile([P, 1], F32, tag="an")
            nc.scalar.activation(an, ansq, ACT.Sqrt)
            dmar = small.tile([P, 1], F32, tag="dmar")
            nc.vector.tensor_sub(dmar, ap, an)
            nc.scalar.activation(losses[:, m:m + 1], dmar, ACT.Relu, bias=1.0)

        # ---- final: sum over 8 chunks then over partitions
        lsum = work.tile([P, 1], F32, tag="lsum")
        nc.vector.tensor_reduce(lsum, losses, axis=AX.X, op=ALU.add)
        ps = psmall.tile([1, 2], F32, tag="pm")
        nc.tensor.matmul(ps, lhsT=lsum, rhs=ones2f, start=True, stop=True)
        outsb = work.tile([1, 1], F32, tag="outsb")
        nc.scalar.copy(outsb, ps[:, 0:1])
        nc.sync.dma_start(out_d, outsb)

    with tile.TileContext(nc) as tc:
        with (
            tc.tile_pool(name="const", bufs=1) as const,
            tc.tile_pool(name="work", bufs=1) as work,
            tc.tile_pool(name="g16p", bufs=4) as g16p,
            tc.tile_pool(name="small", bufs=3) as small,
            tc.tile_pool(name="pbig", bufs=3, space="PSUM") as pbig,
            tc.tile_pool(name="psmall", bufs=2, space="PSUM") as psmall,
        ):
            if reps == 1:
                body(tc, const, work, g16p, small, pbig, psmall)
            else:
                with tc.For_i(0, reps, 1):
                    body(tc, const, work, g16p, small, pbig, psmall)

    nc.compile()
    return nc


def make_in_maps(batch: np.ndarray):
    E = np.ascontiguousarray(batch.reshape(N, D)).astype(np.float16)
    ET = np.ascontiguousarray(E.T)
    idx = np.arange(P)
    same = (idx[:, None] // S) == (idx[None, :] // S)
    # mwin[:, o*TN + c] = +MASKF inside the chunk's 128-col diag window
    # (off = o*128) at same-cluster entries, else 0. Added to g16 = n-2G
    # before the negative min so positives are excluded.
    mwin = np.zeros((P, 4 * TN), np.float16)
    for o in range(4):
        off = o * P
        mwin[:, o * TN + off:o * TN + off + P] = np.where(same, MASKF, 0.0)
    # psel: 0 at same-cluster, -MASKF elsewhere. Added to the diag block
    # before the positive max so non-cluster entries are excluded.
    psel = np.where(same, 0.0, -MASKF).astype(np.float16)
    oneh = np.ones((P, P), np.float16)
    in_maps = []
    for r in range(CORES):
        et_r = np.ascontiguousarray(np.roll(ET, -r * M, axis=1))
        in_maps.append({"et": et_r, "mwin": mwin, "psel": psel,
                        "oneh": oneh,
                        "oner": np.ones((1, P), np.float16),
                        "ones2h": np.ones((P, 2), np.float16),
                        "ones2f": np.ones((P, 2), np.float32)})
    return in_maps


def kernel(batch: np.ndarray) -> np.ndarray:
    if "nc" not in _CACHE:
        _CACHE["nc"] = build_program(reps=1)
    nc = _CACHE["nc"]
    in_maps = make_in_maps(np.asarray(batch))
    res = run_bass_kernel_spmd(nc, in_maps, core_ids=list(range(CORES)))
    total = sum(float(res.results[r]["out"][0, 0]) for r in range(CORES))
    return np.float32(total / N)
